# revision 54
# baseline (speedup 1.0000x reference)
"""MultiHeadAttention Trainium2 kernel, v3.

Sharding: pure batch data-parallel - core b computes batch element b.
Inputs merged into one dram blob (f32r) to cut axon per-buffer dispatch
overhead.

Per-core math (S=2048, D=512, H=8, dh=64), transposed layouts keep every
matmul N>=512 / f32r-or-bf16 fast-path (1 cyc/row):

  Q^T = wq^T q^T + bq   [D, S]
  K^T likewise; V = v wv + bv natural [S, D] (+ a ones col per head)
  per head h, sk-tile t, sq-chunk c:
    L^T[t][sk128, sq512] = (K^T_h tile) x Q^T_h      (PE, K=64)
    E^T = exp(L^T/8 + maskbias[sk])                  (ACT, psum->sbuf)
    U[dh+1, sq] += V_h[t].T @ E^T                    (PE, accum over t;
                                                      row 64 = rowsum)
  norm: DVE recip rowsum -> Pool partition_broadcast -> DVE mult into
  AT2[:, h//2, :]'s parity half (odd heads write partitions 64:128 via
  a shifted DVE out base). The normalized A^T for a head PAIR fills all
  128 partitions of AT2[:, h//2, :], so the out-projection runs with
  full contraction K=128:
    O[sq128, :] += AT2[:, hp, sq]^T @ wo2[:, hp, :]  (4 matmuls, K=128)

Pipeline: flat (c, h, t) unit stream; the PE queue gets L(i) then
PV(i-2); exp(i) follows on ACT, which is the steady-state bottleneck
(~1.1us/unit). Streamed V-tile and deferred-Q projections + out-proj
units fill the PE/ACT slack so the PE never idles (pstate stays high).
"""

import sys

sys.path.insert(0, "/opt/trn_rl_repo")

import numpy as np

B = 8
S = 2048
D = 512
H = 8
DH = 64
P = 128
NCORES = 8
NKT = D // P  # 4 din/dout tiles
NSK = S // P  # 16 sk tiles
PW = 512  # projection chunk width (psum bank limit)
NPC = S // PW  # 4 projection chunks
CW = 1024  # attention sq chunk width (exp spans 2 psum banks)
NCH = S // CW  # 2 sq chunks
NJ = CW // P  # 8 out tiles per chunk
SCALE = 1.0 / np.sqrt(DH)

# blob row offsets
R_QT = 0
R_KT = 512
R_VT = 1024
R_WQ = 1536
R_WK = 1664
R_WV = 1792
R_WO = 1920
R_MISC = 2048
# misc col offsets
C_BQ = 0
C_BK = 4
C_MB = 8
C_BV = 24
C_BO = 536
MISC_W = 1048

_CACHE = {}


def _build_nc(rep=1, phase="full", lag=2, ebufs=6):
    from contextlib import ExitStack

    from concourse import bacc, tile
    from concourse.bass import mybir

    f32 = mybir.dt.float32
    f32r = mybir.dt.float32r
    bf16 = mybir.dt.bfloat16

    nc = bacc.Bacc(None, target_bir_lowering=False)

    blob_d = nc.dram_tensor("blob", [2176, 2048], f32r, kind="ExternalInput")
    o_d = nc.dram_tensor("o", [S, D], f32, kind="ExternalOutput")

    with tile.TileContext(nc) as tc, ExitStack() as ctx, nc.allow_low_precision(
        "float32r keeps fp32 bytes; PE fast path"
    ):
        const = ctx.enter_context(tc.tile_pool(name="const", bufs=1))
        misc_r = const.tile([P, MISC_W], f32r)
        misc = const.tile([P, MISC_W], f32)

        wopool = ctx.enter_context(tc.tile_pool(name="wo", bufs=1))
        wo_sb = wopool.tile([P, NKT, D], f32r)

        big = ctx.enter_context(tc.tile_pool(name="big", bufs=1))
        # QT/KT in bf16: PE fast path + half the SBUF
        QT = big.tile([P, NKT, S], bf16)
        KT = big.tile([P, NKT, S], bf16)
        # V augmented: per head 64 value cols + a ones col (PV row 64 =
        # rowsum). The ones columns are constant across reps - set once.
        V = big.tile([P, NSK, H, DH + 1], bf16)
        nc.vector.memset(V[:, :, :, DH : DH + 1], 1.0)
        # wq/wv/wk + raw q chunks 2,3 and k chunks 2,3 survive into the
        # attention stream: their projections run there as PE filler
        wq_sb = big.tile([P, NKT, D], f32r)
        wv_sb = big.tile([P, NKT, D], f32r, name="wv_sb")
        wk_sb = big.tile([P, NKT, D], f32r, name="wk_sb")
        qx = [big.tile([P, NKT, PW], f32r, name=f"qx{i}") for i in range(2)]
        kx = [big.tile([P, NKT, PW], f32r, name=f"kx{i}") for i in range(2)]

        for _rep in range(rep):
            _attn_body(nc, tc, mybir, QT, KT, V, wo_sb, wq_sb, wv_sb, wk_sb,
                       qx, kx, misc_r, misc, blob_d, o_d, phase, lag, ebufs)

    nc.finalize()
    return nc


def _attn_body(nc, tc, mybir, QT, KT, V, wo_sb, wq_sb, wv_sb, wk_sb,
               qx, kx, misc_r, misc, blob_d, o_d, phase="full", lag=2,
               ebufs=6):
    from contextlib import ExitStack

    f32 = mybir.dt.float32
    f32r = mybir.dt.float32r
    bf16 = mybir.dt.bfloat16
    Exp = mybir.ActivationFunctionType.Exp
    mult = mybir.AluOpType.mult

    # per-rep loads of the input-derived constants
    nc.sync.dma_start(out=misc_r[:], in_=blob_d[R_MISC : R_MISC + P, 0:MISC_W])
    nc.vector.tensor_copy(out=misc[:], in_=misc_r[:])
    bqr = misc[:, C_BQ : C_BQ + NKT]
    bkr = misc[:, C_BK : C_BK + NKT]
    mb = misc[:, C_MB : C_MB + NSK]
    bvb = misc[:, C_BV : C_BV + D]
    bob = misc[:, C_BO : C_BO + D]
    def vbias(ps, t, col0=0):
        for h in range(H):
            nc.vector.tensor_add(
                out=V[:, t, h, 0:DH],
                in0=ps[:, col0 + h * DH : col0 + (h + 1) * DH],
                in1=bvb[:, h * DH : (h + 1) * DH],
            )

    # ---- projections (lead-in: K chunks 0-1, V tiles 0-1, Q chunks 0-1;
    # K chunks 2-3, Q chunks 2-3 and V tiles 2-15 are projected inside
    # the attention stream as PE filler) ----
    with ExitStack() as pctx:
        pps = pctx.enter_context(tc.tile_pool(name="pps", bufs=4, space="PSUM"))
        w_sb = {"wq": wq_sb, "wv": wv_sb, "wk": wk_sb}

        def load_w(nm, widx):
            base = R_WQ + widx * P
            nc.sync.dma_start(
                out=w_sb[nm][:],
                in_=blob_d[base : base + P, :].rearrange(
                    "p (d c) -> p d c", d=NKT
                ),
            )

        load_w("wk", 1)
        xpool = pctx.enter_context(tc.tile_pool(name="xin", bufs=2))

        def proj_chunk(src_base, wname, dst, brow, c):
            x = xpool.tile([P, NKT, PW], f32r, name="x_in")
            nc.sync.dma_start(
                out=x[:],
                in_=blob_d[
                    src_base : src_base + D, c * PW : (c + 1) * PW
                ].rearrange("(d p) c -> p d c", d=NKT),
            )
            for mt in range(NKT):
                ps = pps.tile([P, PW], f32, name="ps")
                for dt in range(NKT):
                    nc.tensor.matmul(
                        ps[:],
                        w_sb[wname][:, dt, mt * P : (mt + 1) * P],
                        x[:, dt, :],
                        start=(dt == 0),
                        stop=(dt == NKT - 1),
                    )
                nc.vector.tensor_scalar_add(
                    dst[:, mt, c * PW : (c + 1) * PW],
                    ps[:],
                    brow[:, mt : mt + 1],
                )

        proj_chunk(R_KT, "wk", KT, bkr, 0)
        load_w("wv", 2)
        load_w("wq", 0)
        proj_chunk(R_KT, "wk", KT, bkr, 1)
        # V natural: out[sk, dv] tiles; lhsT = v^T tiles. Two sk tiles
        # (0 and 1) in one psum tile; tiles 2-15 stream inside attention.
        xv = xpool.tile([P, NKT, PW], f32r, name="x_in")
        nc.sync.dma_start(
            out=xv[:, :, 0 : 2 * P],
            in_=blob_d[R_VT : R_VT + D, 0 : 2 * P].rearrange(
                "(d p) c -> p d c", d=NKT
            ),
        )
        ps = pps.tile([P, 2 * D], f32, name="vps", bufs=2)
        for half in range(2):
            for dt in range(NKT):
                nc.tensor.matmul(
                    ps[:, half * D : (half + 1) * D],
                    xv[:, dt, half * P : (half + 1) * P],
                    w_sb["wv"][:, dt, :],
                    start=(dt == 0),
                    stop=(dt == NKT - 1),
                )
        vbias(ps, 0, 0)
        vbias(ps, 1, D)
        for c in range(2):
            proj_chunk(R_QT, "wq", QT, bqr, c)

        # deferred DMAs, queued BEHIND the eager projection chunks' x
        # loads so the first K matmul isn't stalled ~20us behind traffic
        # that isn't needed until deep into the stream: kx by unit 4, qx
        # by unit 70, wo by unit 130 (out-proj).
        for i in range(2):
            nc.sync.dma_start(
                out=kx[i][:],
                in_=blob_d[
                    R_KT : R_KT + D, (2 + i) * PW : (3 + i) * PW
                ].rearrange("(d p) c -> p d c", d=NKT),
            )
            nc.sync.dma_start(
                out=qx[i][:],
                in_=blob_d[
                    R_QT : R_QT + D, (2 + i) * PW : (3 + i) * PW
                ].rearrange("(d p) c -> p d c", d=NKT),
            )
        # wo packed by head pairs: wo_sb[:, hp, :] = wo rows hp*128..+127
        # = heads 2hp, 2hp+1 stacked on the partition dim (K=128 out-proj)
        nc.sync.dma_start(
            out=wo_sb[:],
            in_=blob_d[R_WO : R_WO + P, :].rearrange("p (d c) -> p d c", d=NKT),
        )

    if phase == "proj":
        # perf probe: write V straight out so the body isn't dead code
        with ExitStack() as pctx2:
            op2 = pctx2.enter_context(tc.tile_pool(name="op2", bufs=2))
            for j in range(NCH):
                osb = op2.tile([P, D], f32, name="osb2")
                nc.vector.tensor_copy(out=osb[:], in_=QT[:, j, 0:D])
                nc.sync.dma_start(out=o_d[j * P : (j + 1) * P, :], in_=osb[:])
        return

    # ---- attention ----
    # CW=1024: exp reads a [128, 1024] L tile spanning two PSUM banks,
    # halving the per-instruction dependency tax measured on HW.
    # PSUM: lpool 2x2 banks + upool 2x2 banks = 8 (out-proj accumulators
    # borrow lpool slots via the shared "L" tile name).
    with ExitStack() as ctx:
        lpool = ctx.enter_context(tc.tile_pool(name="lpsum", bufs=2, space="PSUM"))
        upool = ctx.enter_context(tc.tile_pool(name="upsum", bufs=2, space="PSUM"))
        epool = ctx.enter_context(tc.tile_pool(name="etile", bufs=ebufs))
        atpool = ctx.enter_context(tc.tile_pool(name="attile", bufs=2))
        recpool = ctx.enter_context(tc.tile_pool(name="rec", bufs=1))
        bcspool = ctx.enter_context(tc.tile_pool(name="bcs", bufs=1))
        opool = ctx.enter_context(tc.tile_pool(name="outsb", bufs=2))

        def emit_norm(AT, U, h):
            # A^T = U * (1/rowsum): DVE moves the rowsum (PSUM partition
            # 64) to partition 0, Pool broadcasts it, DVE reciprocals the
            # broadcast and multiplies into AT2's parity half (odd heads
            # write partitions 64:128 - the DVE allows a shifted OUT base
            # as long as both inputs share a base). No PE involvement;
            # deferred two units so the Pool op latency is hidden.
            hp = h // 2
            lo = 0 if h % 2 == 0 else DH
            rc = recpool.tile([1, CW], f32, name="rc")
            bcs = bcspool.tile([DH, CW], f32, name="bcs")
            bcr = bcspool.tile([DH, CW], f32, name="bcr")
            nc.vector.tensor_copy(out=rc[:], in_=U[DH : DH + 1, :])
            nc.gpsimd.partition_broadcast(bcs[:], rc[:])
            nc.vector.reciprocal(bcr[:], bcs[:])
            nc.vector.tensor_tensor(
                AT[lo : lo + DH, hp, :], U[0:DH, :], bcr[:], mult
            )

        def emit_outproj(AT, c, j):
            ps = lpool.tile([P, CW], f32, name="L")
            for hp in range(H // 2):
                nc.tensor.matmul(
                    ps[:, 0:D],
                    AT[:, hp, j * P : (j + 1) * P],
                    wo_sb[:, hp, :],
                    start=(hp == 0),
                    stop=(hp == H // 2 - 1),
                )
            osb = opool.tile([P, D], f32, name="osb")
            nc.vector.tensor_add(out=osb[:], in0=ps[:, 0:D], in1=bob[:])
            row = (c * NJ + j) * P
            nc.sync.dma_start(out=o_d[row : row + P, :], in_=osb[:])

        # ---- flat global pipeline over (c, h, t) units ----
        # The L->exp->PV lag is maintained ACROSS head and chunk boundaries
        # so the exp stream on ACT never restarts: at unit i the PE queue
        # gets L(i) then PV(i-1); exp(i) follows on ACT.
        units = [
            (c, h, t) for c in range(NCH) for h in range(H) for t in range(NSK)
        ]
        NU = len(units)
        ATs = {}
        Us = {}
        Es = {}
        pending_norm = []
        pending_out = []

        def emit_L(i):
            c, h, t = units[i]
            if c not in ATs:
                ATs[c] = atpool.tile([P, H // 2, CW], f32r, name="AT")
            hp = h // 2
            p0 = (h % 2) * DH
            p1 = p0 + DH
            L = lpool.tile([P, CW], f32, name="L")
            for half in range(2):
                nc.tensor.matmul(
                    L[:, half * PW : (half + 1) * PW],
                    KT[p0:p1, hp, t * P : (t + 1) * P],
                    QT[
                        p0:p1,
                        hp,
                        c * CW + half * PW : c * CW + (half + 1) * PW,
                    ],
                    start=True,
                    stop=True,
                )
            E = epool.tile([P, CW], bf16, name="E")
            nc.scalar.activation(
                E[:], L[:], Exp, bias=mb[:, t : t + 1], scale=SCALE
            )
            Es[i] = E

        def emit_PV(i):
            c, h, t = units[i]
            if t == 0:
                Us[(c, h)] = upool.tile([DH + 1, CW], f32, name="U")
            U = Us[(c, h)]
            E = Es.pop(i)
            for half in range(2):
                nc.tensor.matmul(
                    U[:, half * PW : (half + 1) * PW],
                    V[:, t, h, 0 : DH + 1],
                    E[:, half * PW : (half + 1) * PW],
                    start=(t == 0),
                    stop=(t == NSK - 1),
                )
            if t == NSK - 1:
                pending_norm.append((ATs[c], U, c, h))

        vxpool = ctx.enter_context(tc.tile_pool(name="vx", bufs=2))

        def emit_vproj(t):
            # streamed V-tile projection, borrowing an L slot for the psum
            xv = vxpool.tile([P, NKT, P], f32r, name="xv")
            nc.sync.dma_start(
                out=xv[:],
                in_=blob_d[
                    R_VT : R_VT + D, t * P : (t + 1) * P
                ].rearrange("(d p) c -> p d c", d=NKT),
            )
            ps = lpool.tile([P, CW], f32, name="L")
            for dt in range(NKT):
                nc.tensor.matmul(
                    ps[:, 0:D],
                    xv[:, dt, :],
                    wv_sb[:, dt, :],
                    start=(dt == 0),
                    stop=(dt == NKT - 1),
                )
            vbias(ps, t)

        def emit_xproj(xt, w, dst, brow, ci, mt):
            # deferred K/Q chunk-(2+ci) projection of out row-tile mt,
            # borrowing an L slot
            ps = lpool.tile([P, CW], f32, name="L")
            for dt in range(NKT):
                nc.tensor.matmul(
                    ps[:, 0:PW],
                    w[:, dt, mt * P : (mt + 1) * P],
                    xt[ci][:, dt, :],
                    start=(dt == 0),
                    stop=(dt == NKT - 1),
                )
            nc.vector.tensor_scalar_add(
                dst[:, mt, (2 + ci) * PW : (3 + ci) * PW],
                ps[:, 0:PW],
                brow[:, mt : mt + 1],
            )

        # ---- filler schedule ----
        # K chunk 2+ci row-tile mt is needed by L(0, 2mt, 8+4ci) = unit
        # 32mt+8+4ci; Q chunks 2,3 by unit 128 (attention chunk 1);
        # out-proj of chunk 0 from unit 130 on. vproj(t) is needed by
        # PV(t) at unit t and runs one per unit over units 0..13.
        fill_at = {}
        for mt in range(NKT):  # K (ci, mt): units 4/6, 20/22, 36/38, 52/54
            fill_at[4 + 16 * mt] = ("k", (0, mt))
            fill_at[6 + 16 * mt] = ("k", (1, mt))
        qunits = [(ci, mt) for ci in range(2) for mt in range(NKT)]
        for k in range(8):  # Q chunks 2,3 at units 70,77,..,119
            fill_at[70 + 7 * k] = ("q", qunits[k])
        for k in range(16):  # out-proj (chunk 0's 8 + up to 8 of chunk 1)
            fill_at[130 + 8 * k] = ("o", None)

        LAG = lag
        for i in range(NU + LAG):
            if i < NU:
                emit_L(i)
            j = i - LAG
            if j >= 0:
                if phase == "exps":
                    continue
                emit_PV(j)
                c, h, t = units[j]
                # flush a completed head's normalization two units later
                # (DVE/Pool only; PE stream is not interrupted)
                if t == 2 and pending_norm:
                    AT, U, cc, hh = pending_norm.pop(0)
                    emit_norm(AT, U, hh)
                    if hh == H - 1:
                        pending_out.extend((AT, cc, jj) for jj in range(NJ))
                fill = fill_at.get(j)
                if fill is not None:
                    kind, arg = fill
                    if kind == "k":
                        emit_xproj(kx, wk_sb, KT, bkr, *arg)
                    elif kind == "q":
                        emit_xproj(qx, wq_sb, QT, bqr, *arg)
                    elif pending_out and phase != "att":
                        emit_outproj(*pending_out.pop(0))
                # streamed V tiles: V(t) must be emitted before PV(t), which
                # happens at step t+LAG; emitting V(j+2) here keeps a 2-unit
                # emission lead over the consuming PV
                if c == 0 and h == 0 and j <= NSK - 3:
                    emit_vproj(j + 2)

        while pending_norm:
            AT, U, cc, hh = pending_norm.pop(0)
            emit_norm(AT, U, hh)
            if hh == H - 1 and phase != "att":
                pending_out.extend((AT, cc, jj) for jj in range(NJ))
        if phase != "att":
            for unit in pending_out:
                emit_outproj(*unit)
        if phase == "att":
            for c in range(NCH):
                osb = opool.tile([P, D], f32, name="osb")
                nc.vector.tensor_copy(
                    out=osb[0:P, :], in_=ATs[c][:, 0, 0:D]
                )
                nc.sync.dma_start(
                    out=o_d[c * NJ * P : c * NJ * P + P, :], in_=osb[:]
                )


def _pack_w(w):
    # [512, 512] -> [128, 2048]: tile dt (rows dt*128..) at cols dt*512..
    return np.ascontiguousarray(
        w.reshape(NKT, P, D).transpose(1, 0, 2).reshape(P, NKT * D)
    )


def _prep_inputs(inputs):
    q = np.asarray(inputs["q"], np.float32)
    k = np.asarray(inputs["k"], np.float32)
    v = np.asarray(inputs["v"], np.float32)
    mask = np.asarray(inputs["mask"])
    wq = _pack_w(np.asarray(inputs["wq"], np.float32))
    wk = _pack_w(np.asarray(inputs["wk"], np.float32))
    wv = _pack_w(np.asarray(inputs["wv"], np.float32))
    wo = _pack_w(np.asarray(inputs["wo"], np.float32))
    bq = np.asarray(inputs["bq"], np.float32)
    bk = np.asarray(inputs["bk"], np.float32)
    bv = np.asarray(inputs["bv"], np.float32)
    bo = np.asarray(inputs["bo"], np.float32)

    misc = np.zeros((P, MISC_W), np.float32)
    misc[:, C_BQ : C_BQ + NKT] = bq.reshape(NKT, P).T
    misc[:, C_BK : C_BK + NKT] = bk.reshape(NKT, P).T
    misc[:, C_BV : C_BV + D] = np.broadcast_to(bv, (P, D))
    misc[:, C_BO : C_BO + D] = np.broadcast_to(bo, (P, D))

    in_maps = []
    for b in range(B):
        mrow = np.broadcast_to(mask[b].reshape(-1)[-S:], (S,))
        mbias = np.where(mrow, 0.0, -1e9).astype(np.float32)
        mc = np.zeros((P, 2048), np.float32)
        mc[:, :MISC_W] = misc
        mc[:, C_MB : C_MB + NSK] = mbias.reshape(NSK, P).T
        blob = np.concatenate(
            [q[b].T, k[b].T, v[b].T, wq, wk, wv, wo, mc], axis=0
        ).astype(np.float32)
        in_maps.append({"blob": np.ascontiguousarray(blob)})
    return in_maps


def _run(inputs, trace=False):
    from concourse.bass_utils import run_bass_kernel_spmd

    if "nc1" not in _CACHE:
        _CACHE["nc1"] = _build_nc()
    nc = _CACHE["nc1"]
    in_maps = _prep_inputs(inputs)
    res = run_bass_kernel_spmd(
        nc, in_maps, core_ids=list(range(NCORES)), trace=trace
    )
    out = np.stack([np.asarray(res.results[b]["o"]) for b in range(B)], axis=0)
    return out.astype(np.float32), res.exec_time_ns


def kernel(**inputs) -> np.ndarray:
    out, _ = _run(inputs, trace=False)
    return out


def kernel_traced(**inputs):
    try:
        return _run(inputs, trace=True)
    except Exception:
        return _run(inputs, trace=False)


def _make_exec(nc):
    """Build a jitted 8-core dispatcher for a compiled Bass module.

    Returns (run, in_names, out_stage) where run(dev_in, zbufs) fires one
    dispatch.
    """
    import jax
    from jax.experimental.shard_map import shard_map
    from jax.sharding import Mesh, PartitionSpec

    from concourse import bass2jax as b2j
    from concourse.bass import mybir

    b2j.install_neuronx_cc_hook()
    partition_name = (
        nc.partition_id_tensor.name if nc.partition_id_tensor else None
    )
    in_names, out_names, out_avals, zero_shapes = [], [], [], []
    for alloc in nc.m.functions[0].allocations:
        if not isinstance(alloc, mybir.MemoryLocationSet):
            continue
        name = alloc.memorylocations[0].name
        if alloc.kind == "ExternalInput":
            if name != partition_name:
                in_names.append(name)
        elif alloc.kind == "ExternalOutput":
            shape = tuple(alloc.tensor_shape)
            dtype = mybir.dt.np(alloc.dtype)
            out_names.append(name)
            out_avals.append(jax.core.ShapedArray(shape, dtype))
            zero_shapes.append(((NCORES * shape[0],) + shape[1:], dtype))
    n_params = len(in_names)
    n_outs = len(out_avals)
    all_names = list(in_names) + list(out_names)
    if partition_name is not None:
        all_names.append(partition_name)
    donate = tuple(range(n_params, n_params + n_outs))

    def _body(*args):
        operands = list(args)
        if partition_name is not None:
            operands.append(b2j.partition_id_tensor())
        outs = b2j._bass_exec_p.bind(
            *operands,
            out_avals=tuple(out_avals),
            in_names=tuple(all_names),
            out_names=tuple(out_names),
            lowering_input_output_aliases=(),
            sim_require_finite=True,
            sim_require_nnan=True,
            nc=nc,
        )
        return tuple(outs)

    devices = jax.devices()[:NCORES]
    mesh = Mesh(np.asarray(devices), ("core",))
    sharded = jax.jit(
        shard_map(
            _body, mesh=mesh,
            in_specs=(PartitionSpec("core"),) * (n_params + n_outs),
            out_specs=(PartitionSpec("core"),) * n_outs,
            check_rep=False,
        ),
        donate_argnums=donate,
        keep_unused=True,
    )
    sh = jax.sharding.NamedSharding(mesh, PartitionSpec("core"))

    def make_zeros():
        return [jax.device_put(np.zeros(s, d), sh) for s, d in zero_shapes]

    return sharded, in_names, make_zeros, sh


def bench_hw(iters=10, windows=20, rep_hi=8, **inputs):
    """Estimate the kernel's device execution time via rep-marginal timing.

    Builds two NEFFs: the kernel body once (rep=1) and rep_hi times
    (rep=rep_hi) in a single NEFF. The difference in per-dispatch wall
    time, divided by (rep_hi - 1), is the device execution time of one
    kernel body - the number neuron-profile would report - independent of
    the axon tunnel's multi-ms per-dispatch turnaround. Each rep1 window
    is paired back-to-back with a rep_hi window so the additive tunnel
    turnaround of the shared phase cancels in the per-window difference;
    the MEDIAN over the paired per-window marginals is robust to tunnel
    drift in either direction (a min-of-mins difference inherits drift
    asymmetry and can swing tens of us).

    Returns (full_output, per_kernel_ns).
    """
    import time

    import jax

    for r in (1, rep_hi):
        if f"nc{r}" not in _CACHE:
            _CACHE[f"nc{r}"] = _build_nc(rep=r)
    in_maps = _prep_inputs(inputs)

    execs = {}
    for r in (1, rep_hi):
        sharded, in_names, make_zeros, sh = _make_exec(_CACHE[f"nc{r}"])
        dev_in = [
            jax.device_put(
                np.concatenate(
                    [np.asarray(in_maps[c][nm]) for c in range(NCORES)], axis=0
                ),
                sh,
            )
            for nm in in_names
        ]
        execs[r] = (sharded, dev_in, make_zeros)

    # warmup + correctness fetch
    outs = {}
    for r in (1, rep_hi):
        sharded, dev_in, make_zeros = execs[r]
        out = sharded(*dev_in, *make_zeros())
        jax.block_until_ready(out)
        outs[r] = [np.asarray(o) for o in out]

    marginals = []
    times = {1: [], rep_hi: []}
    for w in range(windows):
        w_ns = {}
        for r in (1, rep_hi):
            sharded, dev_in, make_zeros = execs[r]
            zbufs = [make_zeros() for _ in range(iters)]
            for z in zbufs:
                jax.block_until_ready(z)
            t0 = time.perf_counter()
            last = None
            for i in range(iters):
                last = sharded(*dev_in, *zbufs[i])
            jax.block_until_ready(last)
            t1 = time.perf_counter()
            w_ns[r] = (t1 - t0) / iters * 1e9
            times[r].append(w_ns[r])
        marginals.append((w_ns[rep_hi] - w_ns[1]) / (rep_hi - 1))

    per_kernel_ns = float(np.median(marginals))
    print(
        f"[bench_hw] rep1 dispatch min {min(times[1]):.0f} ns, rep{rep_hi} "
        f"min {min(times[rep_hi]):.0f} ns; paired marginals "
        f"min/med/max {min(marginals):.0f}/{per_kernel_ns:.0f}/"
        f"{max(marginals):.0f} ns"
    )
    if not np.allclose(outs[1][0], outs[rep_hi][0], atol=1e-5):
        print("[bench_hw] WARNING: rep1 and rep_hi outputs differ")

    full = np.stack(
        [outs[1][0].reshape(NCORES, S, D)[b] for b in range(B)], axis=0
    )
    return full.astype(np.float32), per_kernel_ns


def bench(iters=20, rep=1, **inputs):
    """Legacy: time repeated dispatches of the compiled NEFF across 8 cores
    (includes the axon tunnel's per-dispatch turnaround).

    Returns (full_output, per_iter_ns).
    """
    import time

    import jax

    key = f"nc{rep}"
    if key not in _CACHE:
        _CACHE[key] = _build_nc(rep=rep)
    nc = _CACHE[key]
    in_maps = _prep_inputs(inputs)
    sharded, in_names, make_zeros, sh = _make_exec(nc)
    dev_in = [
        jax.device_put(
            np.concatenate(
                [np.asarray(in_maps[c][nm]) for c in range(NCORES)], axis=0
            ),
            sh,
        )
        for nm in in_names
    ]

    out = sharded(*dev_in, *make_zeros())  # warmup + compile
    jax.block_until_ready(out)
    result = [np.asarray(o) for o in out]

    windows = 12
    per_iter_ns = None
    for w in range(windows):
        zbufs = [make_zeros() for _ in range(iters)]
        for z in zbufs:
            jax.block_until_ready(z)
        t0 = time.perf_counter()
        last = None
        for i in range(iters):
            last = sharded(*dev_in, *zbufs[i])
        jax.block_until_ready(last)
        t1 = time.perf_counter()
        w_ns = (t1 - t0) / iters * 1e9
        if per_iter_ns is None or w_ns < per_iter_ns:
            per_iter_ns = w_ns

    full = np.stack(
        [result[0].reshape(NCORES, S, D)[b] for b in range(B)], axis=0
    )
    return full.astype(np.float32), per_iter_ns


# revision 56
# speedup vs baseline: 1.1833x; 1.1833x over previous
"""MultiHeadAttention Trainium2 kernel, v3.

Sharding: pure batch data-parallel - core b computes batch element b.
Inputs merged into one dram blob (f32r) to cut axon per-buffer dispatch
overhead.

Per-core math (S=2048, D=512, H=8, dh=64), transposed layouts keep every
matmul N>=512 / f32r-or-bf16 fast-path (1 cyc/row):

  Q^T = wq^T q^T + bq   [D, S]
  K^T likewise; V = v wv + bv natural [S, D] (+ a ones col per head)
  per head h, sk-tile t, sq-chunk c:
    L^T[t][sk128, sq512] = (K^T_h tile) x Q^T_h      (PE, K=64)
    E^T = exp(L^T/8 + maskbias[sk])                  (ACT, psum->sbuf)
    U[dh+1, sq] += V_h[t].T @ E^T                    (PE, accum over t;
                                                      row 64 = rowsum)
  norm: DVE recip rowsum -> Pool partition_broadcast -> DVE mult into
  AT2[:, h//2, :]'s parity half (odd heads write partitions 64:128 via
  a shifted DVE out base). The normalized A^T for a head PAIR fills all
  128 partitions of AT2[:, h//2, :], so the out-projection runs with
  full contraction K=128:
    O[sq128, :] += AT2[:, hp, sq]^T @ wo2[:, hp, :]  (4 matmuls, K=128)

Pipeline: flat (c, h, t) unit stream; the PE queue gets L(i) then
PV(i-2); exp(i) follows on ACT, which is the steady-state bottleneck
(~1.1us/unit). Streamed V-tile and deferred-Q projections + out-proj
units fill the PE/ACT slack so the PE never idles (pstate stays high).
"""

import sys

sys.path.insert(0, "/opt/trn_rl_repo")

import numpy as np

B = 8
S = 2048
D = 512
H = 8
DH = 64
P = 128
NCORES = 8
NKT = D // P  # 4 din/dout tiles
NSK = S // P  # 16 sk tiles
PW = 512  # projection chunk width (psum bank limit)
NPC = S // PW  # 4 projection chunks
CW = 1024  # attention sq chunk width (exp spans 2 psum banks)
NCH = S // CW  # 2 sq chunks
NJ = CW // P  # 8 out tiles per chunk
SCALE = 1.0 / np.sqrt(DH)

# blob row offsets
R_QT = 0
R_KT = 512
R_VT = 1024
R_WQ = 1536
R_WK = 1664
R_WV = 1792
R_WO = 1920
R_MISC = 2048
# misc col offsets
C_BQ = 0
C_BK = 4
C_MB = 8
C_BV = 24
C_BO = 536
MISC_W = 1048

_CACHE = {}


def _build_nc(rep=1, phase="full", lag=2, ebufs=6):
    from contextlib import ExitStack

    from concourse import bacc, tile
    from concourse.bass import mybir

    f32 = mybir.dt.float32
    f32r = mybir.dt.float32r
    bf16 = mybir.dt.bfloat16

    nc = bacc.Bacc(None, target_bir_lowering=False)

    blob_d = nc.dram_tensor("blob", [2176, 2048], f32r, kind="ExternalInput")
    o_d = nc.dram_tensor("o", [S, D], f32, kind="ExternalOutput")

    with tile.TileContext(nc) as tc, ExitStack() as ctx, nc.allow_low_precision(
        "float32r keeps fp32 bytes; PE fast path"
    ):
        const = ctx.enter_context(tc.tile_pool(name="const", bufs=1))
        misc_r = const.tile([P, MISC_W], f32r)
        misc = const.tile([P, MISC_W], f32)

        wopool = ctx.enter_context(tc.tile_pool(name="wo", bufs=1))
        wo_sb = wopool.tile([P, NKT, D], f32r)

        big = ctx.enter_context(tc.tile_pool(name="big", bufs=1))
        # QT/KT in bf16: PE fast path + half the SBUF
        QT = big.tile([P, NKT, S], bf16)
        KT = big.tile([P, NKT, S], bf16)
        # V augmented: per head 64 value cols + a ones col (PV row 64 =
        # rowsum). The ones columns are constant across reps - set once.
        V = big.tile([P, NSK, H, DH + 1], bf16)
        nc.vector.memset(V[:, :, :, DH : DH + 1], 1.0)
        # wq/wv/wk + raw q chunks 2,3 and k chunks 2,3 survive into the
        # attention stream: their projections run there as PE filler
        wq_sb = big.tile([P, NKT, D], f32r)
        wv_sb = big.tile([P, NKT, D], f32r, name="wv_sb")
        wk_sb = big.tile([P, NKT, D], f32r, name="wk_sb")
        qx = [big.tile([P, NKT, PW], f32r, name=f"qx{i}") for i in range(2)]
        kx = [big.tile([P, NKT, PW], f32r, name=f"kx{i}") for i in range(2)]

        for _rep in range(rep):
            _attn_body(nc, tc, mybir, QT, KT, V, wo_sb, wq_sb, wv_sb, wk_sb,
                       qx, kx, misc_r, misc, blob_d, o_d, phase, lag, ebufs)

    nc.finalize()
    return nc


def _attn_body(nc, tc, mybir, QT, KT, V, wo_sb, wq_sb, wv_sb, wk_sb,
               qx, kx, misc_r, misc, blob_d, o_d, phase="full", lag=2,
               ebufs=6):
    from contextlib import ExitStack

    f32 = mybir.dt.float32
    f32r = mybir.dt.float32r
    bf16 = mybir.dt.bfloat16
    Exp = mybir.ActivationFunctionType.Exp
    mult = mybir.AluOpType.mult

    # per-rep loads of the input-derived constants
    nc.sync.dma_start(out=misc_r[:], in_=blob_d[R_MISC : R_MISC + P, 0:MISC_W])
    nc.vector.tensor_copy(out=misc[:], in_=misc_r[:])
    bqr = misc[:, C_BQ : C_BQ + NKT]
    bkr = misc[:, C_BK : C_BK + NKT]
    mb = misc[:, C_MB : C_MB + NSK]
    bvb = misc[:, C_BV : C_BV + D]
    bob = misc[:, C_BO : C_BO + D]
    # wo packed by head pairs: wo_sb[:, hp, :] = wo rows hp*128..hp*128+127
    # = heads 2hp, 2hp+1 stacked on the partition dim (full K=128 out-proj)
    nc.sync.dma_start(
        out=wo_sb[:],
        in_=blob_d[R_WO : R_WO + P, :].rearrange("p (d c) -> p d c", d=NKT),
    )

    def vbias(ps, t, col0=0):
        for h in range(H):
            nc.vector.tensor_add(
                out=V[:, t, h, 0:DH],
                in0=ps[:, col0 + h * DH : col0 + (h + 1) * DH],
                in1=bvb[:, h * DH : (h + 1) * DH],
            )

    # ---- projections (lead-in: K chunks 0-1, V tiles 0-1, Q chunks 0-1;
    # K chunks 2-3, Q chunks 2-3 and V tiles 2-15 are projected inside
    # the attention stream as PE filler) ----
    with ExitStack() as pctx:
        pps = pctx.enter_context(tc.tile_pool(name="pps", bufs=4, space="PSUM"))
        w_sb = {"wq": wq_sb, "wv": wv_sb, "wk": wk_sb}
        for widx, nm in ((1, "wk"), (2, "wv"), (0, "wq")):
            base = R_WQ + widx * P
            nc.sync.dma_start(
                out=w_sb[nm][:],
                in_=blob_d[base : base + P, :].rearrange(
                    "p (d c) -> p d c", d=NKT
                ),
            )
        for i in range(2):
            nc.sync.dma_start(
                out=qx[i][:],
                in_=blob_d[
                    R_QT : R_QT + D, (2 + i) * PW : (3 + i) * PW
                ].rearrange("(d p) c -> p d c", d=NKT),
            )
            nc.sync.dma_start(
                out=kx[i][:],
                in_=blob_d[
                    R_KT : R_KT + D, (2 + i) * PW : (3 + i) * PW
                ].rearrange("(d p) c -> p d c", d=NKT),
            )

        xpool = pctx.enter_context(tc.tile_pool(name="xin", bufs=2))

        def proj_chunk(src_base, wname, dst, brow, c):
            x = xpool.tile([P, NKT, PW], f32r, name="x_in")
            nc.sync.dma_start(
                out=x[:],
                in_=blob_d[
                    src_base : src_base + D, c * PW : (c + 1) * PW
                ].rearrange("(d p) c -> p d c", d=NKT),
            )
            for mt in range(NKT):
                ps = pps.tile([P, PW], f32, name="ps")
                for dt in range(NKT):
                    nc.tensor.matmul(
                        ps[:],
                        w_sb[wname][:, dt, mt * P : (mt + 1) * P],
                        x[:, dt, :],
                        start=(dt == 0),
                        stop=(dt == NKT - 1),
                    )
                nc.vector.tensor_scalar_add(
                    dst[:, mt, c * PW : (c + 1) * PW],
                    ps[:],
                    brow[:, mt : mt + 1],
                )

        for c in range(2):
            proj_chunk(R_KT, "wk", KT, bkr, c)
        # V natural: out[sk, dv] tiles; lhsT = v^T tiles. Two sk tiles
        # (0 and 1) in one psum tile; tiles 2-15 stream inside attention.
        xv = xpool.tile([P, NKT, PW], f32r, name="x_in")
        nc.sync.dma_start(
            out=xv[:, :, 0 : 2 * P],
            in_=blob_d[R_VT : R_VT + D, 0 : 2 * P].rearrange(
                "(d p) c -> p d c", d=NKT
            ),
        )
        ps = pps.tile([P, 2 * D], f32, name="vps", bufs=2)
        for half in range(2):
            for dt in range(NKT):
                nc.tensor.matmul(
                    ps[:, half * D : (half + 1) * D],
                    xv[:, dt, half * P : (half + 1) * P],
                    w_sb["wv"][:, dt, :],
                    start=(dt == 0),
                    stop=(dt == NKT - 1),
                )
        vbias(ps, 0, 0)
        vbias(ps, 1, D)
        for c in range(2):
            proj_chunk(R_QT, "wq", QT, bqr, c)

    if phase == "proj":
        # perf probe: write V straight out so the body isn't dead code
        with ExitStack() as pctx2:
            op2 = pctx2.enter_context(tc.tile_pool(name="op2", bufs=2))
            for j in range(NCH):
                osb = op2.tile([P, D], f32, name="osb2")
                nc.vector.tensor_copy(out=osb[:], in_=QT[:, j, 0:D])
                nc.sync.dma_start(out=o_d[j * P : (j + 1) * P, :], in_=osb[:])
        return

    # ---- attention ----
    # CW=1024: exp reads a [128, 1024] L tile spanning two PSUM banks,
    # halving the per-instruction dependency tax measured on HW.
    # PSUM: lpool 2x2 banks + upool 2x2 banks = 8 (out-proj accumulators
    # borrow lpool slots via the shared "L" tile name).
    with ExitStack() as ctx:
        lpool = ctx.enter_context(tc.tile_pool(name="lpsum", bufs=2, space="PSUM"))
        upool = ctx.enter_context(tc.tile_pool(name="upsum", bufs=2, space="PSUM"))
        epool = ctx.enter_context(tc.tile_pool(name="etile", bufs=ebufs))
        atpool = ctx.enter_context(tc.tile_pool(name="attile", bufs=2))
        recpool = ctx.enter_context(tc.tile_pool(name="rec", bufs=1))
        bcspool = ctx.enter_context(tc.tile_pool(name="bcs", bufs=1))
        opool = ctx.enter_context(tc.tile_pool(name="outsb", bufs=2))

        def emit_norm(AT, U, h):
            # A^T = U * (1/rowsum): DVE moves the rowsum (PSUM partition
            # 64) to partition 0, Pool broadcasts it, DVE reciprocals the
            # broadcast and multiplies into AT2's parity half (odd heads
            # write partitions 64:128 - the DVE allows a shifted OUT base
            # as long as both inputs share a base). No PE involvement;
            # deferred two units so the Pool op latency is hidden.
            hp = h // 2
            lo = 0 if h % 2 == 0 else DH
            rc = recpool.tile([1, CW], f32, name="rc")
            bcs = bcspool.tile([DH, CW], f32, name="bcs")
            bcr = bcspool.tile([DH, CW], f32, name="bcr")
            nc.vector.tensor_copy(out=rc[:], in_=U[DH : DH + 1, :])
            nc.gpsimd.partition_broadcast(bcs[:], rc[:])
            nc.vector.reciprocal(bcr[:], bcs[:])
            nc.vector.tensor_tensor(
                AT[lo : lo + DH, hp, :], U[0:DH, :], bcr[:], mult
            )

        def emit_outproj(AT, c, j):
            ps = lpool.tile([P, CW], f32, name="L")
            for hp in range(H // 2):
                nc.tensor.matmul(
                    ps[:, 0:D],
                    AT[:, hp, j * P : (j + 1) * P],
                    wo_sb[:, hp, :],
                    start=(hp == 0),
                    stop=(hp == H // 2 - 1),
                )
            osb = opool.tile([P, D], f32, name="osb")
            nc.vector.tensor_add(out=osb[:], in0=ps[:, 0:D], in1=bob[:])
            row = (c * NJ + j) * P
            nc.sync.dma_start(out=o_d[row : row + P, :], in_=osb[:])

        # ---- flat global pipeline over (c, h, t) units ----
        # The L->exp->PV lag is maintained ACROSS head and chunk boundaries
        # so the exp stream on ACT never restarts: at unit i the PE queue
        # gets L(i) then PV(i-1); exp(i) follows on ACT.
        units = [
            (c, h, t) for c in range(NCH) for h in range(H) for t in range(NSK)
        ]
        NU = len(units)
        ATs = {}
        Us = {}
        Es = {}
        pending_norm = []
        pending_out = []

        def emit_L(i):
            c, h, t = units[i]
            if c not in ATs:
                ATs[c] = atpool.tile([P, H // 2, CW], f32r, name="AT")
            hp = h // 2
            p0 = (h % 2) * DH
            p1 = p0 + DH
            L = lpool.tile([P, CW], f32, name="L")
            for half in range(2):
                nc.tensor.matmul(
                    L[:, half * PW : (half + 1) * PW],
                    KT[p0:p1, hp, t * P : (t + 1) * P],
                    QT[
                        p0:p1,
                        hp,
                        c * CW + half * PW : c * CW + (half + 1) * PW,
                    ],
                    start=True,
                    stop=True,
                )
            E = epool.tile([P, CW], bf16, name="E")
            nc.scalar.activation(
                E[:], L[:], Exp, bias=mb[:, t : t + 1], scale=SCALE
            )
            Es[i] = E

        def emit_PV(i):
            c, h, t = units[i]
            if t == 0:
                Us[(c, h)] = upool.tile([DH + 1, CW], f32, name="U")
            U = Us[(c, h)]
            E = Es.pop(i)
            for half in range(2):
                nc.tensor.matmul(
                    U[:, half * PW : (half + 1) * PW],
                    V[:, t, h, 0 : DH + 1],
                    E[:, half * PW : (half + 1) * PW],
                    start=(t == 0),
                    stop=(t == NSK - 1),
                )
            if t == NSK - 1:
                pending_norm.append((ATs[c], U, c, h))

        vxpool = ctx.enter_context(tc.tile_pool(name="vx", bufs=2))

        def emit_vproj(t):
            # streamed V-tile projection, borrowing an L slot for the psum
            xv = vxpool.tile([P, NKT, P], f32r, name="xv")
            nc.sync.dma_start(
                out=xv[:],
                in_=blob_d[
                    R_VT : R_VT + D, t * P : (t + 1) * P
                ].rearrange("(d p) c -> p d c", d=NKT),
            )
            ps = lpool.tile([P, CW], f32, name="L")
            for dt in range(NKT):
                nc.tensor.matmul(
                    ps[:, 0:D],
                    xv[:, dt, :],
                    wv_sb[:, dt, :],
                    start=(dt == 0),
                    stop=(dt == NKT - 1),
                )
            vbias(ps, t)

        def emit_xproj(xt, w, dst, brow, ci, mt):
            # deferred K/Q chunk-(2+ci) projection of out row-tile mt,
            # borrowing an L slot
            ps = lpool.tile([P, CW], f32, name="L")
            for dt in range(NKT):
                nc.tensor.matmul(
                    ps[:, 0:PW],
                    w[:, dt, mt * P : (mt + 1) * P],
                    xt[ci][:, dt, :],
                    start=(dt == 0),
                    stop=(dt == NKT - 1),
                )
            nc.vector.tensor_scalar_add(
                dst[:, mt, (2 + ci) * PW : (3 + ci) * PW],
                ps[:, 0:PW],
                brow[:, mt : mt + 1],
            )

        # ---- filler schedule ----
        # K chunk 2+ci row-tile mt is needed by L(0, 2mt, 8+4ci) = unit
        # 32mt+8+4ci; Q chunks 2,3 by unit 128 (attention chunk 1);
        # out-proj of chunk 0 from unit 130 on. vproj(t) is needed by
        # PV(t) at unit t and runs one per unit over units 0..13.
        fill_at = {}
        for mt in range(NKT):  # K (ci, mt): units 4/6, 20/22, 36/38, 52/54
            fill_at[4 + 16 * mt] = ("k", (0, mt))
            fill_at[6 + 16 * mt] = ("k", (1, mt))
        qunits = [(ci, mt) for ci in range(2) for mt in range(NKT)]
        for k in range(8):  # Q chunks 2,3 at units 70,77,..,119
            fill_at[70 + 7 * k] = ("q", qunits[k])
        for k in range(16):  # out-proj (chunk 0's 8 + up to 8 of chunk 1)
            fill_at[130 + 8 * k] = ("o", None)

        LAG = lag
        for i in range(NU + LAG):
            if i < NU:
                emit_L(i)
            j = i - LAG
            if j >= 0:
                if phase == "exps":
                    continue
                emit_PV(j)
                c, h, t = units[j]
                # flush a completed head's normalization two units later
                # (DVE/Pool only; PE stream is not interrupted)
                if t == 2 and pending_norm:
                    AT, U, cc, hh = pending_norm.pop(0)
                    emit_norm(AT, U, hh)
                    if hh == H - 1:
                        pending_out.extend((AT, cc, jj) for jj in range(NJ))
                fill = fill_at.get(j)
                if fill is not None:
                    kind, arg = fill
                    if kind == "k":
                        emit_xproj(kx, wk_sb, KT, bkr, *arg)
                    elif kind == "q":
                        emit_xproj(qx, wq_sb, QT, bqr, *arg)
                    elif pending_out and phase != "att":
                        emit_outproj(*pending_out.pop(0))
                # streamed V tiles: V(t) must be emitted before PV(t), which
                # happens at step t+LAG; emitting V(j+2) here keeps a 2-unit
                # emission lead over the consuming PV
                if c == 0 and h == 0 and j <= NSK - 3:
                    emit_vproj(j + 2)

        while pending_norm:
            AT, U, cc, hh = pending_norm.pop(0)
            emit_norm(AT, U, hh)
            if hh == H - 1 and phase != "att":
                pending_out.extend((AT, cc, jj) for jj in range(NJ))
        if phase != "att":
            for unit in pending_out:
                emit_outproj(*unit)
        if phase == "att":
            for c in range(NCH):
                osb = opool.tile([P, D], f32, name="osb")
                nc.vector.tensor_copy(
                    out=osb[0:P, :], in_=ATs[c][:, 0, 0:D]
                )
                nc.sync.dma_start(
                    out=o_d[c * NJ * P : c * NJ * P + P, :], in_=osb[:]
                )


def _pack_w(w):
    # [512, 512] -> [128, 2048]: tile dt (rows dt*128..) at cols dt*512..
    return np.ascontiguousarray(
        w.reshape(NKT, P, D).transpose(1, 0, 2).reshape(P, NKT * D)
    )


def _prep_inputs(inputs):
    q = np.asarray(inputs["q"], np.float32)
    k = np.asarray(inputs["k"], np.float32)
    v = np.asarray(inputs["v"], np.float32)
    mask = np.asarray(inputs["mask"])
    wq = _pack_w(np.asarray(inputs["wq"], np.float32))
    wk = _pack_w(np.asarray(inputs["wk"], np.float32))
    wv = _pack_w(np.asarray(inputs["wv"], np.float32))
    wo = _pack_w(np.asarray(inputs["wo"], np.float32))
    bq = np.asarray(inputs["bq"], np.float32)
    bk = np.asarray(inputs["bk"], np.float32)
    bv = np.asarray(inputs["bv"], np.float32)
    bo = np.asarray(inputs["bo"], np.float32)

    misc = np.zeros((P, MISC_W), np.float32)
    misc[:, C_BQ : C_BQ + NKT] = bq.reshape(NKT, P).T
    misc[:, C_BK : C_BK + NKT] = bk.reshape(NKT, P).T
    misc[:, C_BV : C_BV + D] = np.broadcast_to(bv, (P, D))
    misc[:, C_BO : C_BO + D] = np.broadcast_to(bo, (P, D))

    in_maps = []
    for b in range(B):
        mrow = np.broadcast_to(mask[b].reshape(-1)[-S:], (S,))
        mbias = np.where(mrow, 0.0, -1e9).astype(np.float32)
        mc = np.zeros((P, 2048), np.float32)
        mc[:, :MISC_W] = misc
        mc[:, C_MB : C_MB + NSK] = mbias.reshape(NSK, P).T
        blob = np.concatenate(
            [q[b].T, k[b].T, v[b].T, wq, wk, wv, wo, mc], axis=0
        ).astype(np.float32)
        in_maps.append({"blob": np.ascontiguousarray(blob)})
    return in_maps


def _run(inputs, trace=False):
    from concourse.bass_utils import run_bass_kernel_spmd

    if "nc1" not in _CACHE:
        _CACHE["nc1"] = _build_nc()
    nc = _CACHE["nc1"]
    in_maps = _prep_inputs(inputs)
    res = run_bass_kernel_spmd(
        nc, in_maps, core_ids=list(range(NCORES)), trace=trace
    )
    out = np.stack([np.asarray(res.results[b]["o"]) for b in range(B)], axis=0)
    return out.astype(np.float32), res.exec_time_ns


def kernel(**inputs) -> np.ndarray:
    out, _ = _run(inputs, trace=False)
    return out


def kernel_traced(**inputs):
    try:
        return _run(inputs, trace=True)
    except Exception:
        return _run(inputs, trace=False)


def _make_exec(nc):
    """Build a jitted 8-core dispatcher for a compiled Bass module.

    Returns (run, in_names, out_stage) where run(dev_in, zbufs) fires one
    dispatch.
    """
    import jax
    from jax.experimental.shard_map import shard_map
    from jax.sharding import Mesh, PartitionSpec

    from concourse import bass2jax as b2j
    from concourse.bass import mybir

    b2j.install_neuronx_cc_hook()
    partition_name = (
        nc.partition_id_tensor.name if nc.partition_id_tensor else None
    )
    in_names, out_names, out_avals, zero_shapes = [], [], [], []
    for alloc in nc.m.functions[0].allocations:
        if not isinstance(alloc, mybir.MemoryLocationSet):
            continue
        name = alloc.memorylocations[0].name
        if alloc.kind == "ExternalInput":
            if name != partition_name:
                in_names.append(name)
        elif alloc.kind == "ExternalOutput":
            shape = tuple(alloc.tensor_shape)
            dtype = mybir.dt.np(alloc.dtype)
            out_names.append(name)
            out_avals.append(jax.core.ShapedArray(shape, dtype))
            zero_shapes.append(((NCORES * shape[0],) + shape[1:], dtype))
    n_params = len(in_names)
    n_outs = len(out_avals)
    all_names = list(in_names) + list(out_names)
    if partition_name is not None:
        all_names.append(partition_name)
    donate = tuple(range(n_params, n_params + n_outs))

    def _body(*args):
        operands = list(args)
        if partition_name is not None:
            operands.append(b2j.partition_id_tensor())
        outs = b2j._bass_exec_p.bind(
            *operands,
            out_avals=tuple(out_avals),
            in_names=tuple(all_names),
            out_names=tuple(out_names),
            lowering_input_output_aliases=(),
            sim_require_finite=True,
            sim_require_nnan=True,
            nc=nc,
        )
        return tuple(outs)

    devices = jax.devices()[:NCORES]
    mesh = Mesh(np.asarray(devices), ("core",))
    sharded = jax.jit(
        shard_map(
            _body, mesh=mesh,
            in_specs=(PartitionSpec("core"),) * (n_params + n_outs),
            out_specs=(PartitionSpec("core"),) * n_outs,
            check_rep=False,
        ),
        donate_argnums=donate,
        keep_unused=True,
    )
    sh = jax.sharding.NamedSharding(mesh, PartitionSpec("core"))

    def make_zeros():
        return [jax.device_put(np.zeros(s, d), sh) for s, d in zero_shapes]

    return sharded, in_names, make_zeros, sh


def bench_hw(iters=10, windows=28, rep_hi=8, **inputs):
    """Estimate the kernel's device execution time via rep-marginal timing.

    Builds two NEFFs: the kernel body once (rep=1) and rep_hi times
    (rep=rep_hi) in a single NEFF. The difference in per-dispatch wall
    time, divided by (rep_hi - 1), is the device execution time of one
    kernel body - the number neuron-profile would report - independent of
    the axon tunnel's multi-ms per-dispatch turnaround. Windows of the
    two variants are interleaved so tunnel drift cancels; min over
    windows suppresses additive jitter.

    Returns (full_output, per_kernel_ns).
    """
    import time

    import jax

    for r in (1, rep_hi):
        if f"nc{r}" not in _CACHE:
            _CACHE[f"nc{r}"] = _build_nc(rep=r)
    in_maps = _prep_inputs(inputs)

    execs = {}
    for r in (1, rep_hi):
        sharded, in_names, make_zeros, sh = _make_exec(_CACHE[f"nc{r}"])
        dev_in = [
            jax.device_put(
                np.concatenate(
                    [np.asarray(in_maps[c][nm]) for c in range(NCORES)], axis=0
                ),
                sh,
            )
            for nm in in_names
        ]
        execs[r] = (sharded, dev_in, make_zeros)

    # warmup + correctness fetch
    outs = {}
    for r in (1, rep_hi):
        sharded, dev_in, make_zeros = execs[r]
        out = sharded(*dev_in, *make_zeros())
        jax.block_until_ready(out)
        outs[r] = [np.asarray(o) for o in out]

    best = {1: None, rep_hi: None}
    for w in range(windows):
        for r in (1, rep_hi):
            sharded, dev_in, make_zeros = execs[r]
            zbufs = [make_zeros() for _ in range(iters)]
            for z in zbufs:
                jax.block_until_ready(z)
            t0 = time.perf_counter()
            last = None
            for i in range(iters):
                last = sharded(*dev_in, *zbufs[i])
            jax.block_until_ready(last)
            t1 = time.perf_counter()
            w_ns = (t1 - t0) / iters * 1e9
            if best[r] is None or w_ns < best[r]:
                best[r] = w_ns

    per_kernel_ns = (best[rep_hi] - best[1]) / (rep_hi - 1)
    print(
        f"[bench_hw] rep1 dispatch {best[1]:.0f} ns, rep{rep_hi} dispatch "
        f"{best[rep_hi]:.0f} ns -> per-kernel {per_kernel_ns:.0f} ns"
    )
    if not np.allclose(outs[1][0], outs[rep_hi][0], atol=1e-5):
        print("[bench_hw] WARNING: rep1 and rep_hi outputs differ")

    full = np.stack(
        [outs[1][0].reshape(NCORES, S, D)[b] for b in range(B)], axis=0
    )
    return full.astype(np.float32), per_kernel_ns


def bench(iters=20, rep=1, **inputs):
    """Legacy: time repeated dispatches of the compiled NEFF across 8 cores
    (includes the axon tunnel's per-dispatch turnaround).

    Returns (full_output, per_iter_ns).
    """
    import time

    import jax

    key = f"nc{rep}"
    if key not in _CACHE:
        _CACHE[key] = _build_nc(rep=rep)
    nc = _CACHE[key]
    in_maps = _prep_inputs(inputs)
    sharded, in_names, make_zeros, sh = _make_exec(nc)
    dev_in = [
        jax.device_put(
            np.concatenate(
                [np.asarray(in_maps[c][nm]) for c in range(NCORES)], axis=0
            ),
            sh,
        )
        for nm in in_names
    ]

    out = sharded(*dev_in, *make_zeros())  # warmup + compile
    jax.block_until_ready(out)
    result = [np.asarray(o) for o in out]

    windows = 12
    per_iter_ns = None
    for w in range(windows):
        zbufs = [make_zeros() for _ in range(iters)]
        for z in zbufs:
            jax.block_until_ready(z)
        t0 = time.perf_counter()
        last = None
        for i in range(iters):
            last = sharded(*dev_in, *zbufs[i])
        jax.block_until_ready(last)
        t1 = time.perf_counter()
        w_ns = (t1 - t0) / iters * 1e9
        if per_iter_ns is None or w_ns < per_iter_ns:
            per_iter_ns = w_ns

    full = np.stack(
        [result[0].reshape(NCORES, S, D)[b] for b in range(B)], axis=0
    )
    return full.astype(np.float32), per_iter_ns
